# revision 12
# baseline (speedup 1.0000x reference)
"""De-stationary causal attention (B=2, L=S=2048, H=8, E=64) on 8 TRN2 cores.

Sharding: the 16 (batch, head) pairs are distributed 2-per-core (cores 0-3
get batch 0, heads 0..7; cores 4-7 get batch 1). Each core runs the same
Bass program on its two pairs.

Per-pair algorithm (scores kept TRANSPOSED: s on partitions, l on free dim):
  ST[s, l]  = K^T_tile.T @ Q^T                       (PE, f32r)
  A[s, l]   = exp(ST * (0.125*tau) + 0.125*delta[s]) (ACT, fused scale+bias)
  diag tile masked with upper-triangular 0/1 mask    (DVE)
  OT[e+1, l] accumulates V_aug.T @ A over s-chunks   (PE; V_aug has a ones
              column, so row 64 of OT carries the softmax denominators)
  epilogue: OT -> SBUF -> PE transpose -> [l, 65] -> divide by sums -> out
"""

import copy
import sys

import numpy as np

try:
    import concourse.bass as bass
except ImportError:  # pragma: no cover
    sys.path.insert(0, "/opt/trn_rl_repo")
    import concourse.bass as bass

import concourse.mybir as mybir
import concourse.tile as tile
from concourse.bass_utils import run_bass_kernel_spmd
from concourse.vector_clock import ScopedClock

B, L, H, E = 2, 2048, 8, 64
N_CORES = 8
PAIRS_PER_CORE = 2
SCALE = 1.0 / np.sqrt(np.float32(E))  # 0.125

f32 = mybir.dt.float32
f32r = mybir.dt.float32r

# ---------------------------------------------------------------------------
# Walrus in this toolchain rejects >1 sync-wait per instruction. Split extra
# waits onto NoOps committed just before the instruction on the same engine.
# ---------------------------------------------------------------------------
_NOP_TEMPLATE = {}


def _make_nop(engine, name):
    if engine not in _NOP_TEMPLATE:
        tmp = bass.Bass()
        _NOP_TEMPLATE[engine] = tmp.engines[engine].nop(nofuse=True).ins
    nop = copy.copy(_NOP_TEMPLATE[engine])
    nop.name = name
    nop.engine = engine
    nop.sync_info = None
    return nop


class SplitWaitTileContext(tile.TileContext):
    _ws_counter = 0

    def _split_waits(self, inst):
        si = inst.sync_info
        if si is None or not si.on_wait or len(si.on_wait) <= 1:
            return []
        if inst.engine == mybir.EngineType.Unassigned:
            return []
        waits = list(si.on_wait)
        inst.sync_info = mybir.SyncInfo(
            on_wait=[waits[0]], on_update=list(si.on_update or [])
        )
        nops = []
        for w in waits[1:]:
            SplitWaitTileContext._ws_counter += 1
            nop = _make_nop(inst.engine, f"I-ws{SplitWaitTileContext._ws_counter}")
            nop.sync_info = mybir.SyncInfo(on_wait=[w], on_update=[])
            nops.append(nop)
        return nops

    def _commit_instruction(self, inst, lazy_reg_writes=True):
        for nop in self._split_waits(inst):
            self._add_instruction(nop)
        super()._commit_instruction(inst, lazy_reg_writes)

    def _drain_and_barrier(self, tick_clock, wait_clock):
        nc = self.nc
        probe = nc.sync.nop(nofuse=True)
        wait_clock.add_sem_waits(
            probe.ins, ScopedClock({None: tick_clock.global_clock})
        )
        waits = list(probe.ins.sync_info.on_wait or []) if probe.ins.sync_info else []
        if len(waits) > 1:
            probe.ins.sync_info.on_wait = [waits[0]]
            handles = {h.num: h for h in self.sems.allocated().values()}
            for w in waits[1:]:
                nop = nc.sync.nop(nofuse=True)
                nop.wait_op(handles[w.id], w.wait_value, "sem-ge")
        nc.sync.drain()

        nc.all_engine_barrier()
        assert self.sems is not None
        popped = nc._tile_sem_poison_stack.pop()
        assert popped is self._sem_poison
        nc.clear_and_free_semaphores(list(self.sems.allocated().values()))
        nc.all_engine_barrier()


# ---------------------------------------------------------------------------
# Program builder
# ---------------------------------------------------------------------------

def build_program(st_dtype=f32r, av_dtype=mybir.dt.bfloat16):
    nc = bass.Bass()
    Exp = mybir.ActivationFunctionType.Exp

    qt = nc.declare_dram_parameter("qt", [PAIRS_PER_CORE, E, L], st_dtype, isOutput=False)
    kt = nc.declare_dram_parameter("kt", [PAIRS_PER_CORE, E, L], st_dtype, isOutput=False)
    vv = nc.declare_dram_parameter("vv", [PAIRS_PER_CORE, L, E], av_dtype, isOutput=False)
    bias_d = nc.declare_dram_parameter("bias_d", [128, L // 128], f32, isOutput=False)
    mask = nc.declare_dram_parameter("mask", [128, 128], av_dtype, isOutput=False)
    ident = nc.declare_dram_parameter("ident", [128, 128], f32, isOutput=False)
    oo = nc.declare_dram_parameter("oo", [PAIRS_PER_CORE, L, E], f32, isOutput=True)

    NT = L // 128  # 16 s-tiles / l-tiles
    NB = L // 512  # 4 OT banks

    with SplitWaitTileContext(nc) as tc:
        with (
            tc.tile_pool(name="const", bufs=1) as constp,
            tc.tile_pool(name="qk", bufs=2) as qkp,
            tc.tile_pool(name="vp", bufs=2) as vp,
            tc.tile_pool(name="ap", bufs=3) as ap_pool,
            tc.tile_pool(name="ep", bufs=2) as ep,
            tc.tile_pool(name="outp", bufs=2) as outp,
            tc.tile_pool(name="st", bufs=1, space="PSUM") as stp,
            tc.tile_pool(name="otp", bufs=1, space="PSUM") as otp,
        ):
            mask_sb = constp.tile([128, 128], av_dtype, tag="mask")
            nc.sync.dma_start(out=mask_sb, in_=mask[:])
            ident_sb = constp.tile([128, 128], f32, tag="ident")
            nc.sync.dma_start(out=ident_sb, in_=ident[:])
            bias_sb = constp.tile([128, NT], f32, tag="bias")
            nc.sync.dma_start(out=bias_sb, in_=bias_d[:])

            for pair in range(PAIRS_PER_CORE):
                # Q^T/K^T duplicated on partitions 0-63 and 64-127 so the
                # k=64 score matmuls can row-pack two s-tiles concurrently
                qt_sb = qkp.tile([2 * E, L], st_dtype, tag="qt")
                nc.sync.dma_start(out=qt_sb[0:E, :], in_=qt[pair])
                nc.sync.dma_start(out=qt_sb[E : 2 * E, :], in_=qt[pair])
                kt_sb = qkp.tile([2 * E, L], st_dtype, tag="kt")
                nc.sync.dma_start(out=kt_sb[0:E, :], in_=kt[pair])
                nc.sync.dma_start(out=kt_sb[E : 2 * E, :], in_=kt[pair])
                # V slab [128, 16, 66]: col 64 of last dim = ones
                v_sb = vp.tile([128, NT, E + 2], av_dtype, tag="v")
                # ones column: fill the whole slab with 1.0 first (contiguous
                # memset), then overwrite cols 0..E-1 with V
                nc.vector.memset(v_sb, 1.0)
                nc.sync.dma_start(
                    out=v_sb[:, :, 0:E],
                    in_=vv[pair].rearrange("(t p) e -> p t e", p=128),
                )

                ot_ps = [
                    otp.tile([E + 1, 512], f32, tag=f"ot{j}", name=f"ot{j}")
                    for j in range(NB)
                ]

                def st_chunks(si):
                    l0 = si * 128
                    base = (l0 // 512) * 512
                    cs = base
                    out = []
                    while cs < L:
                        ce = min(cs + 1024, L)
                        out.append((cs, ce))
                        cs = ce
                    return out

                def emit_st_pair(siA, siB, aA, aB):
                    """Row-packed score matmuls for two s-tiles (partition
                    halves 0-63 / 64-127 of the duplicated Q/K slabs run
                    concurrently on disjoint PE row groups), then exp+mask."""
                    stiles = {}
                    plans = {siA: (0, aA), siB: (E, aB)}
                    subs = sorted(
                        {ss for si in plans for (cs, ce) in st_chunks(si)
                         for ss in range(cs, ce, 512)}
                    )
                    for si, (pbase, _) in plans.items():
                        for ci, (cs, ce) in enumerate(st_chunks(si)):
                            stiles[(si, ci)] = stp.tile(
                                [128, 1024], f32, tag=f"st{pbase // E}",
                                name="st",
                            )
                    # interleave the two tiles' matmuls so the row groups
                    # overlap on the PE
                    for ss in subs:
                        se = ss + 512
                        for si, (pbase, _) in plans.items():
                            l0 = si * 128
                            if se <= l0:
                                continue
                            vs = max(ss, l0)
                            chunks = st_chunks(si)
                            ci = next(
                                i for i, (cs, ce) in enumerate(chunks)
                                if cs <= ss < ce
                            )
                            cs = chunks[ci][0]
                            nc.tensor.matmul(
                                stiles[(si, ci)][:, vs - cs : se - cs],
                                kt_sb[pbase : pbase + E, l0 : l0 + 128],
                                qt_sb[pbase : pbase + E, vs:se],
                                start=True,
                                stop=True,
                            )
                    for si, (pbase, a_si) in plans.items():
                        l0 = si * 128
                        for ci, (cs, ce) in enumerate(st_chunks(si)):
                            vs = max(cs, l0)
                            nc.scalar.activation(
                                out=a_si[:, vs:ce],
                                in_=stiles[(si, ci)][:, vs - cs : ce - cs],
                                func=Exp,
                                bias=bias_sb[:, si : si + 1],
                                scale=1.0,
                            )
                        nc.vector.tensor_mul(
                            a_si[:, l0 : l0 + 128],
                            a_si[:, l0 : l0 + 128],
                            mask_sb,
                        )

                def emit_av(si, a_si):
                    """AV accumulation of s-tile si into the OT banks."""
                    l0 = si * 128
                    for lj in range(l0 // 512, NB):
                        a_lo = max(512 * lj, l0)
                        a_hi = 512 * (lj + 1)
                        nc.tensor.matmul(
                            ot_ps[lj][:, a_lo - 512 * lj : 512],
                            v_sb[:, si, 0 : E + 1],
                            a_si[:, a_lo:a_hi],
                            start=(si == 0),
                            stop=(si == min(4 * lj + 3, NT - 1)),
                        )

                # software pipeline: keep PE one si-PAIR ahead of the AV
                # consumer so it never stalls on ACT's exp
                prev = None
                for sp in range(NT // 2):
                    siA, siB = 2 * sp, 2 * sp + 1
                    aA = ap_pool.tile([128, L], av_dtype, tag="A", name="A")
                    aB = ap_pool.tile([128, L], av_dtype, tag="A", name="A")
                    emit_st_pair(siA, siB, aA, aB)
                    if prev is not None:
                        emit_av(prev[0], prev[2])
                        emit_av(prev[1], prev[3])
                    prev = (siA, siB, aA, aB)
                emit_av(prev[0], prev[2])
                emit_av(prev[1], prev[3])

                # epilogue: transpose + normalize + store
                out_sb = outp.tile([128, NT, E], f32, tag="out")
                for lj in range(NB):
                    ot_sb = ep.tile([E + 1, 512], f32, tag="ot_sb")
                    nc.vector.tensor_copy(ot_sb, ot_ps[lj])
                    for c in range(4):
                        lt = 4 * lj + c  # l-tile index
                        ott = stp.tile([128, 1024], f32, tag="st0", name="ott")
                        nc.tensor.transpose(
                            ott[:, 0 : E + 1],
                            ot_sb[:, c * 128 : (c + 1) * 128],
                            ident_sb[0 : E + 1, 0 : E + 1],
                        )
                        recip = ep.tile([128, 1], f32, tag="recip")
                        nc.vector.reciprocal(recip, ott[:, E : E + 1])
                        nc.vector.tensor_scalar_mul(
                            out_sb[:, lt, :],
                            ott[:, 0:E],
                            recip,
                        )
                nc.sync.dma_start(
                    out=oo[pair].rearrange("(t p) e -> p t e", p=128),
                    in_=out_sb,
                )

    return nc


# ---------------------------------------------------------------------------
# Host-side sharding / unsharding
# ---------------------------------------------------------------------------

def _in_maps(queries, keys, values, tau, delta, st_dtype=f32r,
             av_dtype=mybir.dt.bfloat16):
    np_st = mybir.dt.np(st_dtype)
    np_av = mybir.dt.np(av_dtype)
    mask = np.triu(np.ones((128, 128), dtype=np.float32)).astype(np_av)
    ident = np.eye(128, dtype=np.float32)
    maps = []
    for c in range(N_CORES):
        ps = [2 * c, 2 * c + 1]
        b = ps[0] // H
        hs = [p % H for p in ps]
        qscale = np.float32(SCALE * tau[b, 0])
        qt = np.ascontiguousarray(
            np.stack([queries[b, :, h, :].T * qscale for h in hs])
        ).astype(np_st)
        kt = np.ascontiguousarray(
            np.stack([keys[b, :, h, :].T for h in hs])
        ).astype(np_st)
        vv = np.ascontiguousarray(
            np.stack([values[b, :, h, :] for h in hs])
        ).astype(np_av)
        bias_d = np.ascontiguousarray(
            (SCALE * delta[b]).reshape(L // 128, 128).T
        ).astype(np.float32)
        maps.append(
            {
                "qt": qt,
                "kt": kt,
                "vv": vv,
                "bias_d": bias_d,
                "mask": mask,
                "ident": ident,
            }
        )
    return maps


_CACHED = {}


def run(queries, keys, values, tau, delta, trace=False, st_dtype=f32r,
        av_dtype=mybir.dt.bfloat16):
    key = (str(st_dtype), str(av_dtype))
    if key not in _CACHED:
        _CACHED[key] = build_program(st_dtype, av_dtype)
    nc = _CACHED[key]
    in_maps = _in_maps(
        np.asarray(queries),
        np.asarray(keys),
        np.asarray(values),
        np.asarray(tau),
        np.asarray(delta),
        st_dtype=st_dtype,
        av_dtype=av_dtype,
    )
    res = run_bass_kernel_spmd(
        nc, in_maps, core_ids=list(range(N_CORES)), trace=trace
    )
    out = np.empty((B, L, H, E), dtype=np.float32)
    for c in range(N_CORES):
        o = res.results[c]["oo"]
        for i, p in enumerate([2 * c, 2 * c + 1]):
            out[p // H, :, p % H, :] = o[i]
    return out, res


def kernel(queries, keys, values, tau, delta):
    out, _ = run(queries, keys, values, tau, delta, trace=False)
    return out


# revision 13
# speedup vs baseline: 1.3850x; 1.3850x over previous
"""De-stationary causal attention (B=2, L=S=2048, H=8, E=64) on 8 TRN2 cores.

Sharding: the 16 (batch, head) pairs are distributed 2-per-core (cores 0-3
get batch 0, heads 0..7; cores 4-7 get batch 1). Each core runs the same
Bass program on its two pairs.

Math: logits = (Q K^T) * (tau/sqrt(E)) + delta/sqrt(E), causal softmax, A V.
Host-side folds: Q is pre-scaled by tau/sqrt(E); exp(delta/sqrt(E)) is folded
into V (and into the appended denominator column), because
softmax(x + d)_s = exp(x_s) e^{d_s} / sum_j exp(x_j) e^{d_j}.
So the device only computes exp(q'k) with no bias, letting one ACT call span
a whole 4-bank PSUM group.

Device structure per (b,h) pair, scores kept TRANSPOSED (s on partitions):
  bank-major over 4 output l-blocks of 512; for each bank, groups of 4
  s-tiles: ST[s,l] row-packed on the PE (two k=64 matmuls on partition halves
  run concurrently), one exp over the [128,2048] group, causal mask on diag
  blocks, then AV row-packed into two accumulators (k split 64+64), merged on
  the DVE, PE-transposed, normalized by the denominator column, stored.
"""

import copy
import sys

import numpy as np

try:
    import concourse.bass as bass
except ImportError:  # pragma: no cover
    sys.path.insert(0, "/opt/trn_rl_repo")
    import concourse.bass as bass

import concourse.mybir as mybir
import concourse.tile as tile
from concourse.bass_utils import run_bass_kernel_spmd
from concourse.vector_clock import ScopedClock

B, L, H, E = 2, 2048, 8, 64
N_CORES = 8
PAIRS_PER_CORE = 2
SCALE = 1.0 / np.sqrt(np.float32(E))  # 0.125

f32 = mybir.dt.float32
f32r = mybir.dt.float32r
bf16 = mybir.dt.bfloat16

# ---------------------------------------------------------------------------
# Walrus in this toolchain rejects >1 sync-wait per instruction. Split extra
# waits onto NoOps committed just before the instruction on the same engine.
# ---------------------------------------------------------------------------
_NOP_TEMPLATE = {}


def _make_nop(engine, name):
    if engine not in _NOP_TEMPLATE:
        tmp = bass.Bass()
        _NOP_TEMPLATE[engine] = tmp.engines[engine].nop(nofuse=True).ins
    nop = copy.copy(_NOP_TEMPLATE[engine])
    nop.name = name
    nop.engine = engine
    nop.sync_info = None
    return nop


class SplitWaitTileContext(tile.TileContext):
    _ws_counter = 0

    def _split_waits(self, inst):
        si = inst.sync_info
        if si is None or not si.on_wait or len(si.on_wait) <= 1:
            return []
        if inst.engine == mybir.EngineType.Unassigned:
            return []
        waits = list(si.on_wait)
        inst.sync_info = mybir.SyncInfo(
            on_wait=[waits[0]], on_update=list(si.on_update or [])
        )
        nops = []
        for w in waits[1:]:
            SplitWaitTileContext._ws_counter += 1
            nop = _make_nop(inst.engine, f"I-ws{SplitWaitTileContext._ws_counter}")
            nop.sync_info = mybir.SyncInfo(on_wait=[w], on_update=[])
            nops.append(nop)
        return nops

    def _commit_instruction(self, inst, lazy_reg_writes=True):
        for nop in self._split_waits(inst):
            self._add_instruction(nop)
        super()._commit_instruction(inst, lazy_reg_writes)

    def _drain_and_barrier(self, tick_clock, wait_clock):
        nc = self.nc
        probe = nc.sync.nop(nofuse=True)
        wait_clock.add_sem_waits(
            probe.ins, ScopedClock({None: tick_clock.global_clock})
        )
        waits = list(probe.ins.sync_info.on_wait or []) if probe.ins.sync_info else []
        if len(waits) > 1:
            probe.ins.sync_info.on_wait = [waits[0]]
            handles = {h.num: h for h in self.sems.allocated().values()}
            for w in waits[1:]:
                nop = nc.sync.nop(nofuse=True)
                nop.wait_op(handles[w.id], w.wait_value, "sem-ge")
        nc.sync.drain()

        nc.all_engine_barrier()
        assert self.sems is not None
        popped = nc._tile_sem_poison_stack.pop()
        assert popped is self._sem_poison
        nc.clear_and_free_semaphores(list(self.sems.allocated().values()))
        nc.all_engine_barrier()


# ---------------------------------------------------------------------------
# Program builder (bank-major, fully row-packed)
# ---------------------------------------------------------------------------

def build_program(st_dtype=f32r, av_dtype=f32r):
    nc = bass.Bass()
    Exp = mybir.ActivationFunctionType.Exp

    VW = E + 2  # v row: 64 values + denominator col + pad
    qt = nc.declare_dram_parameter("qt", [PAIRS_PER_CORE, E, L], st_dtype, isOutput=False)
    kt = nc.declare_dram_parameter("kt", [PAIRS_PER_CORE, E, L], st_dtype, isOutput=False)
    vv = nc.declare_dram_parameter("vv", [PAIRS_PER_CORE, L, VW], av_dtype, isOutput=False)
    mask = nc.declare_dram_parameter("mask", [128, 128], av_dtype, isOutput=False)
    ident = nc.declare_dram_parameter("ident", [128, 128], f32, isOutput=False)
    oo = nc.declare_dram_parameter("oo", [PAIRS_PER_CORE, L, E], f32, isOutput=True)

    NT = L // 128  # 16 s-tiles / l-tiles
    NB = L // 512  # 4 OT banks

    with SplitWaitTileContext(nc) as tc:
        with (
            tc.tile_pool(name="const", bufs=1) as constp,
            tc.tile_pool(name="qk", bufs=2) as qkp,
            tc.tile_pool(name="vp", bufs=2) as vp,
            tc.tile_pool(name="ap", bufs=3) as ap_pool,
            tc.tile_pool(name="ep", bufs=2) as ep,
            tc.tile_pool(name="outp", bufs=2) as outp,
            tc.tile_pool(name="st", bufs=1, space="PSUM") as stp,
            tc.tile_pool(name="otp", bufs=1, space="PSUM") as otp,
            tc.tile_pool(name="ottp", bufs=2, space="PSUM") as ottp,
        ):
            mask_sb = constp.tile([128, 128], av_dtype, tag="mask")
            nc.sync.dma_start(out=mask_sb, in_=mask[:])
            ident_sb = constp.tile([128, 128], f32, tag="ident")
            nc.sync.dma_start(out=ident_sb, in_=ident[:])

            for pair in range(PAIRS_PER_CORE):
                # Q^T/K^T duplicated on partitions 0-63 and 64-127 so the
                # k=64 score matmuls can row-pack two s-tiles concurrently
                qt_sb = qkp.tile([2 * E, L], st_dtype, tag="qt")
                nc.sync.dma_start(out=qt_sb[0:E, :], in_=qt[pair])
                nc.sync.dma_start(out=qt_sb[E : 2 * E, :], in_=qt[pair])
                kt_sb = qkp.tile([2 * E, L], st_dtype, tag="kt")
                nc.sync.dma_start(out=kt_sb[0:E, :], in_=kt[pair])
                nc.sync.dma_start(out=kt_sb[E : 2 * E, :], in_=kt[pair])
                # V slab [128, 16, 66]; col 64 carries exp(delta') for the
                # softmax denominator (host-folded), col 65 is padding
                v_sb = vp.tile([128, NT, VW], av_dtype, tag="v")
                nc.sync.dma_start(
                    out=v_sb,
                    in_=vv[pair].rearrange("(t p) e -> p t e", p=128),
                )

                out_sb = outp.tile([128, NT, E], f32, tag="out")

                ot_banks = {}

                def emit_st_group(lj, gi):
                    st = stp.tile([128, 4 * 512], f32, tag="st", name="st")
                    for c in range(4):
                        si = 4 * gi + c
                        off = 128 * c if gi == lj else 0
                        half = (c % 2) * E
                        nc.tensor.matmul(
                            st[:, 512 * c + off : 512 * (c + 1)],
                            kt_sb[half : half + E, si * 128 : si * 128 + 128],
                            qt_sb[half : half + E, 512 * lj + off : 512 * lj + 512],
                            start=True,
                            stop=True,
                        )
                    a_grp = ap_pool.tile(
                        [128, 4 * 512], av_dtype, tag="A", name="A"
                    )
                    nc.scalar.activation(out=a_grp, in_=st, func=Exp, scale=1.0)
                    if gi == lj:
                        for c in range(4):
                            colb = 512 * c + 128 * c
                            nc.vector.tensor_mul(
                                a_grp[:, colb : colb + 128],
                                a_grp[:, colb : colb + 128],
                                mask_sb,
                            )
                    return a_grp

                def emit_av_group(lj, gi, a_grp):
                    ota, otb = ot_banks[lj]
                    for c in range(4):
                        si = 4 * gi + c
                        off = 128 * c if gi == lj else 0
                        first = gi == 0 and c == 0
                        last = gi == lj and c == 3
                        nc.tensor.matmul(
                            ota[:, off:512],
                            v_sb[0:E, si, 0 : E + 1],
                            a_grp[0:E, 512 * c + off : 512 * (c + 1)],
                            start=first,
                            stop=last,
                        )
                        nc.tensor.matmul(
                            otb[:, off:512],
                            v_sb[E : 2 * E, si, 0 : E + 1],
                            a_grp[E : 2 * E, 512 * c + off : 512 * (c + 1)],
                            start=first,
                            stop=last,
                        )

                def emit_epilogue(lj):
                    ota, otb = ot_banks.pop(lj)
                    ot_sb = ep.tile([E + 1, 512], f32, tag="ot_sb", name="ot_sb")
                    nc.vector.tensor_copy(ot_sb, ota)
                    nc.vector.tensor_add(ot_sb, ot_sb, otb)
                    for c in range(4):
                        lt = 4 * lj + c
                        ott = ottp.tile([128, 512], f32, tag="ott", name="ott")
                        nc.tensor.transpose(
                            ott[:, 0 : E + 1],
                            ot_sb[:, c * 128 : (c + 1) * 128],
                            ident_sb[0 : E + 1, 0 : E + 1],
                        )
                        recip = ep.tile([128, 1], f32, tag="recip", name="recip")
                        nc.vector.reciprocal(recip, ott[:, E : E + 1])
                        nc.vector.tensor_scalar_mul(
                            out_sb[:, lt, :], ott[:, 0:E], recip
                        )

                # groups: (lj, gi) — bank lj accumulates s-tiles 0..4lj+3 in
                # groups of 4; gi == lj is the diagonal (partial) group.
                # Software-pipelined: PE stays one group ahead of AV.
                groups = [(lj, gi) for lj in range(NB) for gi in range(lj + 1)]
                prev = None
                for lj, gi in groups:
                    if lj not in ot_banks:
                        ot_banks[lj] = (
                            otp.tile([E + 1, 512], f32, tag="ota", name="ota"),
                            otp.tile([E + 1, 512], f32, tag="otb", name="otb"),
                        )
                    a_grp = emit_st_group(lj, gi)
                    if prev is not None:
                        plj, pgi, pa = prev
                        emit_av_group(plj, pgi, pa)
                        if pgi == plj:  # that was the last group of bank plj
                            emit_epilogue(plj)
                    prev = (lj, gi, a_grp)
                plj, pgi, pa = prev
                emit_av_group(plj, pgi, pa)
                emit_epilogue(plj)

                nc.sync.dma_start(
                    out=oo[pair].rearrange("(t p) e -> p t e", p=128),
                    in_=out_sb,
                )

    return nc


# ---------------------------------------------------------------------------
# Host-side sharding / unsharding
# ---------------------------------------------------------------------------

def _in_maps(queries, keys, values, tau, delta, st_dtype=f32r, av_dtype=f32r):
    np_st = mybir.dt.np(st_dtype)
    np_av = mybir.dt.np(av_dtype)
    mask = np.triu(np.ones((128, 128), dtype=np.float32)).astype(np_av)
    ident = np.eye(128, dtype=np.float32)
    maps = []
    for c in range(N_CORES):
        ps = [2 * c, 2 * c + 1]
        b = ps[0] // H
        hs = [p % H for p in ps]
        qscale = np.float32(SCALE * tau[b, 0])
        qt = np.ascontiguousarray(
            np.stack([queries[b, :, h, :].T * qscale for h in hs])
        ).astype(np_st)
        kt = np.ascontiguousarray(
            np.stack([keys[b, :, h, :].T for h in hs])
        ).astype(np_st)
        # V augmented with the delta fold: cols 0..63 = V * exp(delta'),
        # col 64 = exp(delta') (denominator), col 65 pad
        expd = np.exp(SCALE * delta[b]).astype(np.float32)  # [L]
        vv = np.zeros((PAIRS_PER_CORE, L, E + 2), dtype=np.float32)
        for i, h in enumerate(hs):
            vv[i, :, 0:E] = values[b, :, h, :] * expd[:, None]
            vv[i, :, E] = expd
        vv = np.ascontiguousarray(vv).astype(np_av)
        maps.append(
            {"qt": qt, "kt": kt, "vv": vv, "mask": mask, "ident": ident}
        )
    return maps


_CACHED = {}


def run(queries, keys, values, tau, delta, trace=False, st_dtype=f32r,
        av_dtype=f32r):
    key = (str(st_dtype), str(av_dtype))
    if key not in _CACHED:
        _CACHED[key] = build_program(st_dtype, av_dtype)
    nc = _CACHED[key]
    in_maps = _in_maps(
        np.asarray(queries),
        np.asarray(keys),
        np.asarray(values),
        np.asarray(tau),
        np.asarray(delta),
        st_dtype=st_dtype,
        av_dtype=av_dtype,
    )
    res = run_bass_kernel_spmd(
        nc, in_maps, core_ids=list(range(N_CORES)), trace=trace
    )
    out = np.empty((B, L, H, E), dtype=np.float32)
    for c in range(N_CORES):
        o = res.results[c]["oo"]
        for i, p in enumerate([2 * c, 2 * c + 1]):
            out[p // H, :, p % H, :] = o[i]
    return out, res


def kernel(queries, keys, values, tau, delta):
    out, _ = run(queries, keys, values, tau, delta, trace=False)
    return out


# revision 15
# speedup vs baseline: 1.6019x; 1.1566x over previous
"""De-stationary causal attention (B=2, L=S=2048, H=8, E=64) on 8 TRN2 cores.

Sharding: the 16 (batch, head) pairs are distributed 2-per-core (cores 0-3
get batch 0, heads 0..7; cores 4-7 get batch 1). Each core runs the same
Bass program on its two pairs.

Math: logits = (Q K^T) * (tau/sqrt(E)) + delta/sqrt(E), causal softmax, A V.
Host-side folds: Q is pre-scaled by tau/sqrt(E); exp(delta/sqrt(E)) is folded
into V (and into the appended denominator column), because
softmax(x + d)_s = exp(x_s) e^{d_s} / sum_j exp(x_j) e^{d_j}.
So the device only computes exp(q'k) with no bias, letting one ACT call span
a whole 4-bank PSUM group.

Device structure per (b,h) pair, scores kept TRANSPOSED (s on partitions):
  bank-major over 4 output l-blocks of 512; for each bank, groups of 4
  s-tiles: ST[s,l] row-packed on the PE (two k=64 matmuls on partition halves
  run concurrently), one exp over the [128,2048] group, causal mask on diag
  blocks, then AV row-packed into two accumulators (k split 64+64), merged on
  the DVE, PE-transposed, normalized by the denominator column, stored.
"""

import copy
import sys

import numpy as np

try:
    import concourse.bass as bass
except ImportError:  # pragma: no cover
    sys.path.insert(0, "/opt/trn_rl_repo")
    import concourse.bass as bass

import concourse.mybir as mybir
import concourse.tile as tile
from concourse.bass_utils import run_bass_kernel_spmd
from concourse.vector_clock import ScopedClock

B, L, H, E = 2, 2048, 8, 64
N_CORES = 8
PAIRS_PER_CORE = 2
SCALE = 1.0 / np.sqrt(np.float32(E))  # 0.125

f32 = mybir.dt.float32
f32r = mybir.dt.float32r
bf16 = mybir.dt.bfloat16

# ---------------------------------------------------------------------------
# Walrus in this toolchain rejects >1 sync-wait per instruction. Split extra
# waits onto NoOps committed just before the instruction on the same engine.
# ---------------------------------------------------------------------------
_NOP_TEMPLATE = {}


def _make_nop(engine, name):
    if engine not in _NOP_TEMPLATE:
        tmp = bass.Bass()
        _NOP_TEMPLATE[engine] = tmp.engines[engine].nop(nofuse=True).ins
    nop = copy.copy(_NOP_TEMPLATE[engine])
    nop.name = name
    nop.engine = engine
    nop.sync_info = None
    return nop


class SplitWaitTileContext(tile.TileContext):
    _ws_counter = 0

    def _split_waits(self, inst):
        si = inst.sync_info
        if si is None or not si.on_wait or len(si.on_wait) <= 1:
            return []
        if inst.engine == mybir.EngineType.Unassigned:
            return []
        waits = list(si.on_wait)
        inst.sync_info = mybir.SyncInfo(
            on_wait=[waits[0]], on_update=list(si.on_update or [])
        )
        nops = []
        for w in waits[1:]:
            SplitWaitTileContext._ws_counter += 1
            nop = _make_nop(inst.engine, f"I-ws{SplitWaitTileContext._ws_counter}")
            nop.sync_info = mybir.SyncInfo(on_wait=[w], on_update=[])
            nops.append(nop)
        return nops

    def _commit_instruction(self, inst, lazy_reg_writes=True):
        for nop in self._split_waits(inst):
            self._add_instruction(nop)
        super()._commit_instruction(inst, lazy_reg_writes)

    def _drain_and_barrier(self, tick_clock, wait_clock):
        nc = self.nc
        probe = nc.sync.nop(nofuse=True)
        wait_clock.add_sem_waits(
            probe.ins, ScopedClock({None: tick_clock.global_clock})
        )
        waits = list(probe.ins.sync_info.on_wait or []) if probe.ins.sync_info else []
        if len(waits) > 1:
            probe.ins.sync_info.on_wait = [waits[0]]
            handles = {h.num: h for h in self.sems.allocated().values()}
            for w in waits[1:]:
                nop = nc.sync.nop(nofuse=True)
                nop.wait_op(handles[w.id], w.wait_value, "sem-ge")
        nc.sync.drain()

        nc.all_engine_barrier()
        assert self.sems is not None
        popped = nc._tile_sem_poison_stack.pop()
        assert popped is self._sem_poison
        nc.clear_and_free_semaphores(list(self.sems.allocated().values()))
        nc.all_engine_barrier()


# ---------------------------------------------------------------------------
# Program builder (bank-major, fully row-packed)
# ---------------------------------------------------------------------------

def build_program(st_dtype=f32r, av_dtype=f32r):
    nc = bass.Bass()
    Exp = mybir.ActivationFunctionType.Exp

    VW = E + 2  # v row: 64 values + denominator col + pad
    qt = nc.declare_dram_parameter("qt", [PAIRS_PER_CORE, E, L], st_dtype, isOutput=False)
    kt = nc.declare_dram_parameter("kt", [PAIRS_PER_CORE, E, L], st_dtype, isOutput=False)
    vv = nc.declare_dram_parameter("vv", [PAIRS_PER_CORE, L, VW], av_dtype, isOutput=False)
    mask = nc.declare_dram_parameter("mask", [128, 128], av_dtype, isOutput=False)
    ident = nc.declare_dram_parameter("ident", [128, 128], f32, isOutput=False)
    oo = nc.declare_dram_parameter("oo", [PAIRS_PER_CORE, L, E], f32, isOutput=True)

    NT = L // 128  # 16 s-tiles / l-tiles
    NB = L // 512  # 4 OT banks

    with SplitWaitTileContext(nc) as tc:
        with (
            tc.tile_pool(name="const", bufs=1) as constp,
            tc.tile_pool(name="qk", bufs=2) as qkp,
            tc.tile_pool(name="vp", bufs=2) as vp,
            tc.tile_pool(name="ap", bufs=3) as ap_pool,
            tc.tile_pool(name="ep", bufs=2) as ep,
            tc.tile_pool(name="outp", bufs=2) as outp,
            tc.tile_pool(name="st", bufs=1, space="PSUM") as stp,
            tc.tile_pool(name="otp", bufs=1, space="PSUM") as otp,
            tc.tile_pool(name="ottp", bufs=2, space="PSUM") as ottp,
        ):
            mask_sb = constp.tile([128, 128], av_dtype, tag="mask")
            nc.sync.dma_start(out=mask_sb, in_=mask[:])
            ident_sb = constp.tile([128, 128], f32, tag="ident")
            nc.sync.dma_start(out=ident_sb, in_=ident[:])

            for pair in range(PAIRS_PER_CORE):
                # Q^T/K^T duplicated on partitions 0-63 and 64-127 so the
                # k=64 score matmuls can row-pack two s-tiles concurrently
                qt_sb = qkp.tile([2 * E, L], st_dtype, tag="qt")
                kt_sb = qkp.tile([2 * E, L], st_dtype, tag="kt")
                # V slab [128, 16, 66]; col 64 carries exp(delta') for the
                # softmax denominator (host-folded), col 65 is padding
                v_sb = vp.tile([128, NT, VW], av_dtype, tag="v")
                vv_r = vv[pair].rearrange("(t p) e -> p t e", p=128)
                # chunked loads (512 cols / 4 t-rows at a time) so the first
                # score group starts long before the full slabs land
                for ch in range(4):
                    cl = slice(512 * ch, 512 * (ch + 1))
                    nc.sync.dma_start(out=kt_sb[0:E, cl], in_=kt[pair][:, cl])
                    nc.sync.dma_start(
                        out=kt_sb[E : 2 * E, cl], in_=kt[pair][:, cl]
                    )
                    nc.sync.dma_start(out=qt_sb[0:E, cl], in_=qt[pair][:, cl])
                    nc.sync.dma_start(
                        out=qt_sb[E : 2 * E, cl], in_=qt[pair][:, cl]
                    )
                    nc.sync.dma_start(
                        out=v_sb[:, 4 * ch : 4 * ch + 4, :],
                        in_=vv_r[:, 4 * ch : 4 * ch + 4, :],
                    )

                out_sb = outp.tile([128, NT, E], f32, tag="out")

                ot_banks = {}

                def emit_st_group(lj, gi):
                    a_grp = ap_pool.tile(
                        [128, 4 * 512], av_dtype, tag="A", name="A"
                    )
                    for hb in range(2):  # two double-buffered [128,1024] halves
                        st = stp.tile(
                            [128, 1024], f32, tag=f"st{hb}", name="st"
                        )
                        for cc in range(2):
                            c = 2 * hb + cc
                            si = 4 * gi + c
                            off = 128 * c if gi == lj else 0
                            half = (c % 2) * E
                            nc.tensor.matmul(
                                st[:, 512 * cc + off : 512 * (cc + 1)],
                                kt_sb[half : half + E, si * 128 : si * 128 + 128],
                                qt_sb[half : half + E, 512 * lj + off : 512 * lj + 512],
                                start=True,
                                stop=True,
                            )
                        nc.scalar.activation(
                            out=a_grp[:, 1024 * hb : 1024 * (hb + 1)],
                            in_=st,
                            func=Exp,
                            scale=1.0,
                        )
                    if gi == lj:
                        for c in range(4):
                            colb = 512 * c + 128 * c
                            nc.vector.tensor_mul(
                                a_grp[:, colb : colb + 128],
                                a_grp[:, colb : colb + 128],
                                mask_sb,
                            )
                    return a_grp

                def emit_av_group(lj, gi, a_grp):
                    ota, otb = ot_banks[lj]
                    for c in range(4):
                        si = 4 * gi + c
                        off = 128 * c if gi == lj else 0
                        first = gi == 0 and c == 0
                        last = gi == lj and c == 3
                        nc.tensor.matmul(
                            ota[:, off:512],
                            v_sb[0:E, si, 0 : E + 1],
                            a_grp[0:E, 512 * c + off : 512 * (c + 1)],
                            start=first,
                            stop=last,
                        )
                        nc.tensor.matmul(
                            otb[:, off:512],
                            v_sb[E : 2 * E, si, 0 : E + 1],
                            a_grp[E : 2 * E, 512 * c + off : 512 * (c + 1)],
                            start=first,
                            stop=last,
                        )

                def emit_epilogue(lj):
                    ota, otb = ot_banks.pop(lj)
                    ot_sb = ep.tile([E + 1, 512], f32, tag="ot_sb", name="ot_sb")
                    nc.vector.tensor_copy(ot_sb, ota)
                    nc.vector.tensor_add(ot_sb, ot_sb, otb)
                    for c in range(4):
                        lt = 4 * lj + c
                        ott = ottp.tile([128, 512], f32, tag="ott", name="ott")
                        nc.tensor.transpose(
                            ott[:, 0 : E + 1],
                            ot_sb[:, c * 128 : (c + 1) * 128],
                            ident_sb[0 : E + 1, 0 : E + 1],
                        )
                        recip = ep.tile([128, 1], f32, tag="recip", name="recip")
                        nc.vector.reciprocal(recip, ott[:, E : E + 1])
                        nc.vector.tensor_scalar_mul(
                            out_sb[:, lt, :], ott[:, 0:E], recip
                        )

                # groups: (lj, gi) — bank lj accumulates s-tiles 0..4lj+3 in
                # groups of 4; gi == lj is the diagonal (partial) group.
                # Software-pipelined: PE stays one group ahead of AV.
                groups = [(lj, gi) for lj in range(NB) for gi in range(lj + 1)]
                prev = None
                for lj, gi in groups:
                    if lj not in ot_banks:
                        ot_banks[lj] = (
                            otp.tile([E + 1, 512], f32, tag="ota", name="ota"),
                            otp.tile([E + 1, 512], f32, tag="otb", name="otb"),
                        )
                    a_grp = emit_st_group(lj, gi)
                    if prev is not None:
                        plj, pgi, pa = prev
                        emit_av_group(plj, pgi, pa)
                        if pgi == plj:  # that was the last group of bank plj
                            emit_epilogue(plj)
                    prev = (lj, gi, a_grp)
                plj, pgi, pa = prev
                emit_av_group(plj, pgi, pa)
                emit_epilogue(plj)

                nc.sync.dma_start(
                    out=oo[pair].rearrange("(t p) e -> p t e", p=128),
                    in_=out_sb,
                )

    return nc


# ---------------------------------------------------------------------------
# Host-side sharding / unsharding
# ---------------------------------------------------------------------------

def _in_maps(queries, keys, values, tau, delta, st_dtype=f32r, av_dtype=f32r):
    np_st = mybir.dt.np(st_dtype)
    np_av = mybir.dt.np(av_dtype)
    mask = np.triu(np.ones((128, 128), dtype=np.float32)).astype(np_av)
    ident = np.eye(128, dtype=np.float32)
    maps = []
    for c in range(N_CORES):
        ps = [2 * c, 2 * c + 1]
        b = ps[0] // H
        hs = [p % H for p in ps]
        qscale = np.float32(SCALE * tau[b, 0])
        qt = np.ascontiguousarray(
            np.stack([queries[b, :, h, :].T * qscale for h in hs])
        ).astype(np_st)
        kt = np.ascontiguousarray(
            np.stack([keys[b, :, h, :].T for h in hs])
        ).astype(np_st)
        # V augmented with the delta fold: cols 0..63 = V * exp(delta'),
        # col 64 = exp(delta') (denominator), col 65 pad
        expd = np.exp(SCALE * delta[b]).astype(np.float32)  # [L]
        vv = np.zeros((PAIRS_PER_CORE, L, E + 2), dtype=np.float32)
        for i, h in enumerate(hs):
            vv[i, :, 0:E] = values[b, :, h, :] * expd[:, None]
            vv[i, :, E] = expd
        vv = np.ascontiguousarray(vv).astype(np_av)
        maps.append(
            {"qt": qt, "kt": kt, "vv": vv, "mask": mask, "ident": ident}
        )
    return maps


_CACHED = {}


def run(queries, keys, values, tau, delta, trace=False, st_dtype=f32r,
        av_dtype=f32r):
    key = (str(st_dtype), str(av_dtype))
    if key not in _CACHED:
        _CACHED[key] = build_program(st_dtype, av_dtype)
    nc = _CACHED[key]
    in_maps = _in_maps(
        np.asarray(queries),
        np.asarray(keys),
        np.asarray(values),
        np.asarray(tau),
        np.asarray(delta),
        st_dtype=st_dtype,
        av_dtype=av_dtype,
    )
    res = run_bass_kernel_spmd(
        nc, in_maps, core_ids=list(range(N_CORES)), trace=trace
    )
    out = np.empty((B, L, H, E), dtype=np.float32)
    for c in range(N_CORES):
        o = res.results[c]["oo"]
        for i, p in enumerate([2 * c, 2 * c + 1]):
            out[p // H, :, p % H, :] = o[i]
    return out, res


def kernel(queries, keys, values, tau, delta):
    out, _ = run(queries, keys, values, tau, delta, trace=False)
    return out
